# revision 1
# baseline (speedup 1.0000x reference)
"""Trainium2 Bass kernel for nn_AttentionTSSA (B=8, N=8192, C=512, H=8).

Sharding: data-parallel over batch B across the 8 NeuronCores (1 batch each,
no collectives).  Inside each core the computation is:

  phase 1: wT[c, n] = Wqkv @ x^T (fp32r matmuls, chunks of 512 tokens),
           kept resident in SBUF; per-channel norm^2 accumulated on the fly
           via bn_stats on the PSUM tiles.
  phase 2: per 128-token tile, a small matmul with the squared w tile as the
           stationary operand against a [c,16] mixing matrix A produces
           (sum_ws | r) in token-partition layout.  Softmax over the 8 head
           columns (free axis), Pi and Pi*r accumulated into global S / PR
           sums with a ones-matmul.
           The chunk tail expands Pi back to channel layout (PE indicator
           matmul) and scales wT in place; -attn = -1/(1+dots) is later
           folded into WoutT.
  phase 3: pure GEMM stream: outT = WoutT_scaled.T @ wT_scaled, bias fused
           into the PSUM->SBUF evacuation, DMA out transposed.

Host side transposes x per batch (channels must sit on SBUF partitions for
the tensor engine) and un-transposes the per-core outputs.
"""

import numpy as np

B, N, C, H = 8, 8192, 512, 8
D = C // H          # 64
CT = C // 128       # 4 channel tiles
NCH = N // 512      # 16 chunks of 512 tokens
TPC = 4             # token tiles per chunk
NT = N // 128       # 64 token tiles
MASK_NEG = -30.0    # stand-in for the reference's -1e9 (softmax-shift safe)

_CACHE = {}


def _build_bass(reps=1, phases=(1, 2, 3), expand="mm", tune=True):
    import concourse.bacc as bacc
    import concourse.bass as bass
    import concourse.mybir as mybir
    import concourse.tile as tile

    f32 = mybir.dt.float32
    f32r = mybir.dt.float32r
    Alu = mybir.AluOpType
    Act = mybir.ActivationFunctionType

    nc = bacc.Bacc("TRN2", target_bir_lowering=False, debug=False, num_devices=B)

    xT = nc.dram_tensor("xT", [C, N], f32r, kind="ExternalInput")
    wqkvT = nc.dram_tensor("wqkvT", [C, C], f32r, kind="ExternalInput")
    woutT = nc.dram_tensor("woutT", [C, C], f32r, kind="ExternalInput")
    boutT = nc.dram_tensor("boutT", [128, CT], f32, kind="ExternalInput")
    maskf = nc.dram_tensor("maskf", [128, NT], f32, kind="ExternalInput")
    mbias8 = nc.dram_tensor("mbias8", [128, NT], f32, kind="ExternalInput")
    tempP = nc.dram_tensor("tempP", [128, CT], f32, kind="ExternalInput")
    ident = nc.dram_tensor("ident", [128, 128], f32, kind="ExternalInput")
    ind8 = nc.dram_tensor("ind8", [H, C], f32r, kind="ExternalInput")
    outT = nc.dram_tensor("outT", [C, N], f32, kind="ExternalOutput")

    def r(ap):
        return ap.bitcast(f32r)

    with tile.TileContext(nc) as tc:
        with (
            tc.tile_pool(name="singles", bufs=1) as sing,
            tc.tile_pool(name="work", bufs=3) as work,
            tc.tile_pool(name="work1", bufs=1) as work1,
            tc.tile_pool(name="work2", bufs=2) as work2,
            tc.tile_pool(name="ps_big", bufs=4, space="PSUM") as ps_big,
            tc.tile_pool(name="ps_med", bufs=2, space="PSUM") as ps_med,
            tc.tile_pool(name="ps_x", bufs=2, space="PSUM") as ps_x,
        ):
            # ---------------- constants / persistent tiles ----------------
            wq = [sing.tile([128, C], f32r, tag=f"wq{i}", name=f"wq{i}") for i in range(CT)]
            wo = [sing.tile([128, C], f32r, tag=f"wo{i}", name=f"wo{i}") for i in range(CT)]
            wt = [sing.tile([128, N], f32r, tag=f"wt{i}", name=f"wt{i}") for i in range(CT)]
            for i in range(CT):
                nc.sync.dma_start(out=wq[i][:], in_=wqkvT[i * 128:(i + 1) * 128, :])
                nc.sync.dma_start(out=wo[i][:], in_=woutT[i * 128:(i + 1) * 128, :])
            bout_sb = sing.tile([128, CT], f32, tag="bout", name="bout")
            nc.sync.dma_start(out=bout_sb[:], in_=boutT[:])
            maskf_sb = sing.tile([128, NT], f32, tag="maskf", name="maskf")
            nc.sync.dma_start(out=maskf_sb[:], in_=maskf[:])
            mbias_sb = sing.tile([128, NT], f32, tag="mbias", name="mbias")
            nc.sync.dma_start(out=mbias_sb[:], in_=mbias8[:])
            tempP_sb = sing.tile([128, CT], f32, tag="tempP", name="tempP")
            nc.sync.dma_start(out=tempP_sb[:], in_=tempP[:])
            ident_sb = sing.tile([128, 128], f32, tag="ident", name="ident")
            nc.sync.dma_start(out=ident_sb[:], in_=ident[:])
            ind8_sb = sing.tile([H, C], f32r, tag="ind8", name="ind8")
            nc.sync.dma_start(out=ind8_sb[:], in_=ind8[:])

            ones1f = sing.tile([128, 1], f32, tag="ones1f", name="ones1f")
            nc.vector.memset(ones1f[:], 1.0)
            ones1 = sing.tile([128, 1], f32r, tag="ones1", name="ones1")
            nc.vector.tensor_copy(ones1[:], ones1f[:])

            bn_st = [sing.tile([128, NCH, 6], f32, tag=f"bn{i}", name=f"bn{i}") for i in range(CT)]
            pi_all = sing.tile([128, NCH, TPC, H], f32r, tag="pi_all", name="pi_all")
            amat = [sing.tile([128, 16], f32r, tag=f"amat{i}", name=f"amat{i}") for i in range(CT)]
            inv = [sing.tile([128, 1], f32, tag=f"inv{i}", name=f"inv{i}") for i in range(CT)]
            spr = sing.tile([1, 16], f32, tag="spr", name="spr")
            watn = sing.tile([1, H], f32, tag="watn", name="watn")
            watnT = sing.tile([H, 1], f32, tag="watnT", name="watnT")

            # S/PR accumulator psum bank: cols [g2, ti4, h8]
            psS_box = [None]
            if not tune:
                psS_box[0] = ps_x.tile([1, 2 * TPC * H], f32, tag="x", name="psS")

            def phase1():
                for k in range(NCH):
                    psA = []
                    for ci in range(CT):
                        xt = work.tile([128, 512], f32r, tag="xt", name="xt")
                        nc.sync.dma_start(
                            out=xt[:],
                            in_=xT[ci * 128:(ci + 1) * 128, k * 512:(k + 1) * 512])
                        for co in range(CT):
                            if ci == 0:
                                psA.append(ps_big.tile([128, 512], f32, tag="big", name="big"))
                            nc.tensor.matmul(
                                psA[co][:], r(wq[ci][:, co * 128:(co + 1) * 128]), r(xt[:]),
                                start=(ci == 0), stop=(ci == CT - 1),
                            )
                    for co in range(CT):
                        nc.scalar.activation(
                            out=wt[co][:, k * 512:(k + 1) * 512], in_=psA[co][:], func=Act.Copy)
                        if tune:
                            nc.vector.bn_stats(
                                bn_st[co][:, k, :],
                                wt[co][:, k * 512:(k + 1) * 512].bitcast(f32))
                        else:
                            nc.vector.bn_stats(bn_st[co][:, k, :], psA[co][:])

            def norm_finalize():
                for ci in range(CT):
                    mv = work2.tile([128, 2], f32, tag="mv", name="mv")
                    nc.vector.bn_aggr(mv[:], bn_st[ci][:])
                    nsq = work2.tile([128, 1], f32, tag="nsq_f", name="nsq_f")
                    # normsq = N * (var + mean^2); inv = 1 / max(normsq, 1e-24)
                    nc.vector.scalar_tensor_tensor(
                        out=nsq[:], in0=mv[:, 0:1], scalar=mv[:, 1:2], in1=mv[:, 0:1],
                        op0=Alu.bypass, op1=Alu.mult)
                    nc.vector.tensor_scalar_add(nsq[:], nsq[:], mv[:, 1:2])
                    nc.vector.tensor_scalar_max(nsq[:], nsq[:], 1e-24 / N)
                    nc.vector.reciprocal(inv[ci][:], nsq[:])
                    nc.vector.tensor_scalar_mul(inv[ci][:], inv[ci][:], 1.0 / N)
                    am = work2.tile([128, 16], f32, tag="am_f", name="am_f")
                    nc.vector.memset(am[:], 0.0)
                    # col 2ci (rows 0:64) / col 2ci+1 (rows 64:128): inv * temp
                    nc.vector.tensor_copy(am[0:64, 2 * ci:2 * ci + 1], inv[ci][0:64, :])
                    nc.vector.tensor_copy(am[64:128, 2 * ci + 1:2 * ci + 2], inv[ci][64:128, :])
                    nc.vector.tensor_scalar_mul(
                        am[:, 0:H], am[:, 0:H], tempP_sb[:, ci:ci + 1])
                    nc.vector.memset(am[0:64, 8 + 2 * ci:8 + 2 * ci + 1], 1.0)
                    nc.vector.memset(am[64:128, 8 + 2 * ci + 1:8 + 2 * ci + 2], 1.0)
                    nc.vector.tensor_copy(amat[ci][:], am[:])

            def phase2():
                if tune:
                    # per-rep S/PR accumulator in a big-pool slot: phase1 keeps
                    # all 4 banks, and the x pool serves psT double-buffered.
                    psS_box[0] = ps_big.tile([1, 2 * TPC * H], f32, tag="big", name="psS")
                psS = psS_box[0]
                for k in range(NCH):
                    w2c = [work2.tile([128, 512], f32r, tag=f"w2op{i}", name=f"w2c{i}")
                           for i in range(CT)]
                    for ci in range(CT):
                        nc.scalar.activation(
                            out=w2c[ci][:], in_=wt[ci][:, k * 512:(k + 1) * 512],
                            func=Act.Square)
                    lg = work2.tile([128, TPC, H], f32, tag="lg", name="lg")
                    ee = lg
                    erec = work2.tile([128, TPC], f32, tag="erec", name="erec")
                    rc = work2.tile([128, TPC, H], f32, tag="rc", name="rc")
                    for ti in range(TPC):
                        t = k * TPC + ti
                        psB = ps_med.tile([128, 16], f32, tag="medB", name="medB")
                        for ci in range(CT):
                            nc.tensor.matmul(
                                psB[:], r(w2c[ci][:, ti * 128:(ti + 1) * 128]),
                                r(amat[ci][:]),
                                start=(ci == 0), stop=(ci == CT - 1))
                        # logits = sum_ws*mask + mbias*temp (temp folded into amat/mbias)
                        nc.vector.tensor_scalar(
                            out=lg[:, ti, :], in0=psB[:, 0:H],
                            scalar1=maskf_sb[:, t:t + 1], scalar2=mbias_sb[:, t:t + 1],
                            op0=Alu.mult, op1=Alu.add)
                        nc.scalar.activation(out=rc[:, ti, :], in_=psB[:, 8:16], func=Act.Copy)
                    nc.scalar.activation(out=ee[:], in_=lg[:], func=Act.Exp)  # in-place
                    nc.vector.reduce_sum(erec[:], ee[:], axis=mybir.AxisListType.X)
                    nc.vector.reciprocal(erec[:], erec[:])
                    pirt = work2.tile([128, TPC, H], f32r, tag="pirt", name="pirt")
                    for ti in range(TPC):
                        nc.vector.tensor_scalar_mul(
                            pi_all[:, k, ti, :], ee[:, ti, :], erec[:, ti:ti + 1])
                        nc.vector.tensor_mul(
                            pirt[:, ti, :], pi_all[:, k, ti, :], rc[:, ti, :])
                    # single accumulation group covering the whole psS bank:
                    # cols 0:32 <- Pi sums, cols 32:64 <- Pi*r sums.
                    nc.tensor.matmul(
                        psS[0:1, 0:TPC * H], r(ones1[:]), r(pi_all[:, k, :, :]),
                        start=(k == 0), stop=False)
                    nc.tensor.matmul(
                        psS[0:1, TPC * H:2 * TPC * H], r(ones1[:]), r(pirt[:]),
                        start=False, stop=(k == NCH - 1))
                    # expand Pi back to channel layout and scale wT in place:
                    # wt[c, n] *= Pi[h(c), n]  (attn is folded into WoutT later)
                    psT = ps_x.tile([H, 512], f32, tag="x", name="psT")
                    for ti in range(TPC):
                        nc.tensor.transpose(
                            psT[:, ti * 128:(ti + 1) * 128],
                            pi_all[:, k, ti, :].bitcast(f32), ident_sb[:])
                    pitc = work2.tile([H, 512], f32r, tag="pitc", name="pitc")
                    nc.scalar.activation(out=pitc[:], in_=psT[:], func=Act.Copy)
                    for ci in range(CT):
                        # broadcast Pi rows {2ci, 2ci+1} over the tile's 128
                        # channel partitions, then scale wT in place.
                        if expand == "dma":
                            pexp = work2.tile([128, 512], f32r, tag="pexp", name="pexp")
                            src = pitc[2 * ci:2 * ci + 2, :]
                            bsrc = bass.AP(tensor=src.tensor, offset=src.offset,
                                           ap=[src.ap[0], [0, 64], src.ap[1]])
                            nc.sync.dma_start(out=pexp[:], in_=bsrc)
                            nc.vector.tensor_mul(
                                wt[ci][:, k * 512:(k + 1) * 512],
                                wt[ci][:, k * 512:(k + 1) * 512], pexp[:])
                        else:
                            psE = ps_big.tile([128, 512], f32, tag="big", name="bigE")
                            nc.tensor.matmul(
                                psE[:], r(ind8_sb[:, ci * 128:(ci + 1) * 128]), r(pitc[:]),
                                start=True, stop=True)
                            nc.vector.tensor_mul(
                                wt[ci][:, k * 512:(k + 1) * 512],
                                wt[ci][:, k * 512:(k + 1) * 512], psE[:])

            def global_scalars():
                psS = psS_box[0]
                # spr[0,0:8] = S[h], spr[0,8:16] = PR[h]
                nc.vector.reduce_sum(
                    spr[:].rearrange("p (g h) -> p g h", g=2),
                    psS[:].rearrange("p (g t h) -> p g h t", g=2, t=TPC, h=H),
                    axis=mybir.AxisListType.X)
                srec = work2.tile([1, H], f32, tag="srec", name="srec")
                nc.vector.tensor_scalar_add(srec[:], spr[0:1, 0:H], 1e-8)
                nc.vector.reciprocal(srec[:], srec[:])
                dots = work2.tile([1, H], f32, tag="dots", name="dots")
                nc.vector.tensor_mul(dots[:], spr[0:1, H:2 * H], srec[:])
                nc.vector.tensor_scalar_add(dots[:], dots[:], 1.0)
                nc.vector.reciprocal(watn[:], dots[:])
                nc.vector.tensor_scalar_mul(watn[:], watn[:], -1.0)
                psW = ps_med.tile([H, 1], f32, tag="medB", name="psW")
                nc.tensor.matmul(psW[:], watn[:], ident_sb[0:1, 0:1], is_transpose=True)
                nc.scalar.activation(out=watnT[:], in_=psW[:], func=Act.Copy)
                # wo[ci] *= watn[h(c)] (per-partition), via a tiny expand matmul
                for ci in range(CT):
                    psWE = ps_med.tile([128, 1], f32, tag="medB", name="psWE")
                    nc.tensor.matmul(
                        psWE[:], ind8_sb[:, ci * 128:(ci + 1) * 128].bitcast(f32),
                        watnT[:], start=True, stop=True)
                    wex = work2.tile([128, 1], f32, tag="wex", name="wex")
                    nc.scalar.activation(out=wex[:], in_=psWE[:], func=Act.Copy)
                    nc.vector.tensor_scalar_mul(wo[ci][:], wo[ci][:], wex[:])

            def phase3():
                for k in range(NCH):
                    for oj in range(CT):
                        psC = ps_big.tile([128, 512], f32, tag="big", name="bigC")
                        for ci in range(CT):
                            nc.tensor.matmul(
                                psC[:], r(wo[ci][:, oj * 128:(oj + 1) * 128]),
                                r(wt[ci][:, k * 512:(k + 1) * 512]),
                                start=(ci == 0), stop=(ci == CT - 1))
                        oc = work2.tile([128, 512], f32, tag="outc", name="outc")
                        if oj == CT - 1:
                            nc.vector.tensor_scalar_add(oc[:], psC[:], bout_sb[:, oj:oj + 1])
                        else:
                            nc.scalar.activation(
                                out=oc[:], in_=psC[:], func=Act.Identity,
                                bias=bout_sb[:, oj:oj + 1], scale=1.0)
                        nc.sync.dma_start(
                            out=outT[oj * 128:(oj + 1) * 128, k * 512:(k + 1) * 512],
                            in_=oc[:])

            for _rep in range(reps):
                if 1 in phases:
                    phase1()
                    norm_finalize()
                if 2 in phases:
                    phase2()
                if 3 in phases:
                    global_scalars()
                    phase3()

    nc.compile()
    return nc


def _prep_inputs(x, token_mask, Wqkv, temp, Wout, bout):
    f = np.float32
    temp = np.asarray(temp, dtype=f)
    wqkvT = np.ascontiguousarray(np.asarray(Wqkv, f).T)
    woutT = np.ascontiguousarray(np.asarray(Wout, f).T)
    boutT = np.ascontiguousarray(np.asarray(bout, f).reshape(CT, 128).T)
    ident = np.eye(128, dtype=f)
    ind8 = (np.arange(C) // D == np.arange(H)[:, None]).astype(f)
    # tempP[p, ci] = temp[2ci + (p>=64)]
    tempP = np.empty((128, CT), f)
    for ci in range(CT):
        tempP[0:64, ci] = temp[2 * ci, 0]
        tempP[64:128, ci] = temp[2 * ci + 1, 0]
    in_maps = []
    for b in range(B):
        m = np.asarray(token_mask[b], f)          # [N]
        mt = m.reshape(NT, 128).T.copy()          # [128, NT]
        # per-partition mask bias; exact when temp == ones (the fixed fill)
        mb = (mt - 1.0) * (-MASK_NEG)
        in_maps.append({
            "xT": np.ascontiguousarray(np.asarray(x[b], f).T),
            "wqkvT": wqkvT,
            "woutT": woutT,
            "boutT": boutT,
            "maskf": mt,
            "mbias8": np.ascontiguousarray(mb),
            "tempP": tempP,
            "ident": ident,
            "ind8": ind8,
        })
    return in_maps


def kernel(**inputs):
    from concourse.bass_utils import run_bass_kernel_spmd

    if "nc" not in _CACHE:
        _CACHE["nc"] = _build_bass()
    nc = _CACHE["nc"]
    in_maps = _prep_inputs(**inputs)
    try:
        res = run_bass_kernel_spmd(nc, in_maps, core_ids=list(range(B)))
    except Exception:
        # transient device/tunnel hiccup: retry once
        import time as _t
        _t.sleep(2.0)
        res = run_bass_kernel_spmd(nc, in_maps, core_ids=list(range(B)))
    out = np.empty((B, N, C), np.float32)
    for b in range(B):
        out[b] = res.results[b]["outT"].T
    return out



# revision 5
# speedup vs baseline: 1.8503x; 1.8503x over previous
"""Trainium2 Bass kernel for nn_AttentionTSSA (B=8, N=8192, C=512, H=8).

Sharding: data-parallel over batch B across the 8 NeuronCores (1 batch each,
no collectives).  bf16 data path throughout (inputs/outputs quantized on the
host); all matmul accumulation and the norm/softmax reductions stay f32.

Per core, three PE-paced stages with engine-balanced helpers:

  stage 1: wT[c,n] = Wqkv @ x^T in 512-token chunks (bf16 matmuls, f32 PSUM).
           Act evacuates PSUM -> wt (bf16); DVE tensor_tensor_reduce squares
           the same PSUM tile into w2 (bf16) AND emits the per-channel
           norm^2 partial in one pass.  DMA-in (bf16) on the SP queue.
  stage 2: per 128-token tile, a [c,16]-column matmul with the w2 tile as
           stationary produces (sum_ws | r) token-major.  Act computes
           exp(mask*logits + mbias) straight from PSUM (mask folded via
           scale/bias APs), DVE finishes the head-softmax; Pi and Pi*r are
           accumulated into the global S/PR bank with ones-matmuls.  Pi is
           transposed (PE) and broadcast to channel layout with a 0-stride
           SBUF DMA, then DVE scales wt in place (all bf16).
  stage 3: outT = Wout_scaled.T @ wt_scaled as a pure GEMM stream; -1/(1+dots)
           is folded into the wo weights.  Act/DVE alternate on the biased
           PSUM evacuation; DMA-out (bf16) on SP.

Host side: x is transposed/cast to bf16 per batch; outputs are cast back and
un-transposed.
"""

import numpy as np

B, N, C, H = 8, 8192, 512, 8
D = C // H          # 64
CT = C // 128       # 4 channel tiles
NCH = N // 512      # 16 chunks of 512 tokens
TPC = 4             # token tiles per chunk
NT = N // 128       # 64 token tiles
MASK_NEG = -30.0    # stand-in for the reference's -1e9 (softmax-shift safe)

_CACHE = {}


def _build_bass(reps=1, phases=(1, 2, 3)):
    import concourse.bacc as bacc
    import concourse.bass as bass
    import concourse.mybir as mybir
    import concourse.tile as tile

    f32 = mybir.dt.float32
    bf16 = mybir.dt.bfloat16
    Alu = mybir.AluOpType
    Act = mybir.ActivationFunctionType

    nc = bacc.Bacc("TRN2", target_bir_lowering=False, debug=False, num_devices=B)

    xbf = nc.dram_tensor("xbf", [C, N], bf16, kind="ExternalInput")
    wqkvb = nc.dram_tensor("wqkvb", [C, C], bf16, kind="ExternalInput")
    woutb = nc.dram_tensor("woutb", [C, C], bf16, kind="ExternalInput")
    boutT = nc.dram_tensor("boutT", [128, CT], f32, kind="ExternalInput")
    maskf = nc.dram_tensor("maskf", [128, NT], f32, kind="ExternalInput")
    mbias8 = nc.dram_tensor("mbias8", [128, NT], f32, kind="ExternalInput")
    tempP = nc.dram_tensor("tempP", [128, CT], f32, kind="ExternalInput")
    identb = nc.dram_tensor("identb", [128, 128], bf16, kind="ExternalInput")
    ind8b = nc.dram_tensor("ind8b", [H, C], bf16, kind="ExternalInput")
    outT = nc.dram_tensor("outT", [C, N], bf16, kind="ExternalOutput")

    with tile.TileContext(nc) as tc:
        with (
            tc.tile_pool(name="singles", bufs=1) as sing,
            tc.tile_pool(name="small", bufs=2) as small,
        ):
            # ---------------- constants / persistent tiles ----------------
            wq = [sing.tile([128, C], bf16, tag=f"wq{i}", name=f"wq{i}") for i in range(CT)]
            wo = [sing.tile([128, C], bf16, tag=f"wo{i}", name=f"wo{i}") for i in range(CT)]
            wt = [sing.tile([128, N], bf16, tag=f"wt{i}", name=f"wt{i}") for i in range(CT)]
            w2 = [sing.tile([128, N], bf16, tag=f"w2_{i}", name=f"w2_{i}") for i in range(CT)]
            for i in range(CT):
                nc.sync.dma_start(out=wq[i][:], in_=wqkvb[i * 128:(i + 1) * 128, :])
                nc.sync.dma_start(out=wo[i][:], in_=woutb[i * 128:(i + 1) * 128, :])
            bout_sb = sing.tile([128, CT], f32, tag="bout", name="bout")
            nc.sync.dma_start(out=bout_sb[:], in_=boutT[:])
            maskf_sb = sing.tile([128, NT], f32, tag="maskf", name="maskf")
            nc.sync.dma_start(out=maskf_sb[:], in_=maskf[:])
            mbias_sb = sing.tile([128, NT], f32, tag="mbias", name="mbias")
            nc.sync.dma_start(out=mbias_sb[:], in_=mbias8[:])
            tempP_sb = sing.tile([128, CT], f32, tag="tempP", name="tempP")
            nc.sync.dma_start(out=tempP_sb[:], in_=tempP[:])
            ident_sb = sing.tile([128, 128], bf16, tag="ident", name="ident")
            nc.sync.dma_start(out=ident_sb[:], in_=identb[:])
            ind8_sb = sing.tile([H, C], bf16, tag="ind8", name="ind8")
            nc.sync.dma_start(out=ind8_sb[:], in_=ind8b[:])

            ones1 = sing.tile([128, 1], bf16, tag="ones1", name="ones1")
            nc.vector.memset(ones1[:], 1.0)
            idf = sing.tile([1, 1], f32, tag="idf", name="idf")
            nc.vector.memset(idf[:], 1.0)

            nrm = [sing.tile([128, NCH], f32, tag=f"nrm{i}", name=f"nrm{i}") for i in range(CT)]
            pi_all = sing.tile([128, NCH, TPC, H], bf16, tag="pi_all", name="pi_all")
            amat = [sing.tile([128, 16], bf16, tag=f"amat{i}", name=f"amat{i}") for i in range(CT)]
            inv = [sing.tile([128, 1], f32, tag=f"inv{i}", name=f"inv{i}") for i in range(CT)]
            spr = sing.tile([1, 16], f32, tag="spr", name="spr")
            watn = sing.tile([1, H], f32, tag="watn", name="watn")
            watnT = sing.tile([H, 1], bf16, tag="watnT", name="watnT")

            def phase1():
                with (
                    tc.tile_pool(name="p1x", bufs=8) as xp,
                    tc.tile_pool(name="p1ps", bufs=6, space="PSUM") as psp,
                ):
                    for k in range(NCH):
                        xt = []
                        for ci in range(CT):
                            t = xp.tile([128, 512], bf16, tag="xt", name="xt")
                            nc.sync.dma_start(
                                out=t[:],
                                in_=xbf[ci * 128:(ci + 1) * 128, k * 512:(k + 1) * 512])
                            xt.append(t)
                        for co in range(CT):
                            psA = psp.tile([128, 512], f32, tag="psA", name="psA")
                            for ci in range(CT):
                                nc.tensor.matmul(
                                    psA[:], wq[ci][:, co * 128:(co + 1) * 128], xt[ci][:],
                                    start=(ci == 0), stop=(ci == CT - 1))
                            nc.scalar.activation(
                                out=wt[co][:, k * 512:(k + 1) * 512], in_=psA[:],
                                func=Act.Copy)
                            # w2 = psA^2 (bf16) and norm^2 partial in one DVE pass
                            nc.vector.tensor_tensor_reduce(
                                out=w2[co][:, k * 512:(k + 1) * 512],
                                in0=psA[:], in1=psA[:], scale=1.0, scalar=0.0,
                                op0=Alu.mult, op1=Alu.add,
                                accum_out=nrm[co][:, k:k + 1])

            def norm_finalize():
                for ci in range(CT):
                    nsq = small.tile([128, 1], f32, tag="nsq_f", name="nsq_f")
                    nc.vector.reduce_sum(nsq[:], nrm[ci][:], axis=mybir.AxisListType.X)
                    nc.vector.tensor_scalar_max(nsq[:], nsq[:], 1e-24)
                    nc.vector.reciprocal(inv[ci][:], nsq[:])
                    am = small.tile([128, 16], f32, tag="am_f", name="am_f")
                    nc.vector.memset(am[:], 0.0)
                    # col 2ci (rows 0:64) / col 2ci+1 (rows 64:128): inv * temp
                    nc.vector.tensor_copy(am[0:64, 2 * ci:2 * ci + 1], inv[ci][0:64, :])
                    nc.vector.tensor_copy(am[64:128, 2 * ci + 1:2 * ci + 2], inv[ci][64:128, :])
                    nc.vector.tensor_scalar_mul(
                        am[:, 0:H], am[:, 0:H], tempP_sb[:, ci:ci + 1])
                    nc.vector.memset(am[0:64, 8 + 2 * ci:8 + 2 * ci + 1], 1.0)
                    nc.vector.memset(am[64:128, 8 + 2 * ci + 1:8 + 2 * ci + 2], 1.0)
                    nc.vector.tensor_copy(amat[ci][:], am[:])

            def phase2():
                with (
                    tc.tile_pool(name="p2w", bufs=2) as wp,
                    tc.tile_pool(name="p2psB", bufs=3, space="PSUM") as psb,
                    tc.tile_pool(name="p2psT", bufs=2, space="PSUM") as pst,
                    tc.tile_pool(name="p2psS", bufs=1, space="PSUM") as pss,
                ):
                    psS = pss.tile([1, 2 * TPC * H], f32, tag="psS", name="psS")
                    for k in range(NCH):
                        ee = wp.tile([128, TPC, H], f32, tag="ee", name="ee")
                        rc = wp.tile([128, TPC, H], bf16, tag="rc", name="rc")
                        psB = []
                        for ti in range(TPC):
                            t = k * TPC + ti
                            pb = psb.tile([128, 16], f32, tag="psB", name="psB")
                            psB.append(pb)
                            for ci in range(CT):
                                nc.tensor.matmul(
                                    pb[:], w2[ci][:, t * 128:(t + 1) * 128],
                                    amat[ci][:],
                                    start=(ci == 0), stop=(ci == CT - 1))
                            # ee = exp(maskf * sum_ws + mbias), masked -> exp(-30)
                            nc.scalar.activation(
                                out=ee[:, ti, :], in_=pb[:, 0:H], func=Act.Exp,
                                bias=mbias_sb[:, t:t + 1], scale=maskf_sb[:, t:t + 1])
                            nc.scalar.activation(
                                out=rc[:, ti, :], in_=pb[:, 8:16], func=Act.Copy)
                        erec = wp.tile([128, TPC], f32, tag="erec", name="erec")
                        nc.vector.reduce_sum(erec[:], ee[:], axis=mybir.AxisListType.X)
                        nc.vector.reciprocal(erec[:], erec[:])
                        pirt = wp.tile([128, TPC, H], bf16, tag="pirt", name="pirt")
                        for ti in range(TPC):
                            nc.vector.tensor_scalar_mul(
                                pi_all[:, k, ti, :], ee[:, ti, :], erec[:, ti:ti + 1])
                            nc.vector.tensor_mul(
                                pirt[:, ti, :], pi_all[:, k, ti, :], rc[:, ti, :])
                        # global S / PR accumulators: one bank, two column groups
                        nc.tensor.matmul(
                            psS[0:1, 0:TPC * H], ones1[:], pi_all[:, k, :, :],
                            start=(k == 0), stop=(k == NCH - 1))
                        nc.tensor.matmul(
                            psS[0:1, TPC * H:2 * TPC * H], ones1[:], pirt[:],
                            start=(k == 0), stop=(k == NCH - 1))
                        # Pi back to channel layout: PE transpose + broadcast DMA
                        psT = pst.tile([H, 512], bf16, tag="psT", name="psT")
                        for ti in range(TPC):
                            nc.tensor.transpose(
                                psT[:, ti * 128:(ti + 1) * 128],
                                pi_all[:, k, ti, :], ident_sb[:])
                        pitc = wp.tile([H, 512], bf16, tag="pitc", name="pitc")
                        nc.scalar.activation(out=pitc[:], in_=psT[:], func=Act.Copy)
                        for ci in range(CT):
                            pexp = wp.tile([128, 512], bf16, tag="pexp", name="pexp")
                            src = pitc[2 * ci:2 * ci + 2, :]
                            bsrc = bass.AP(tensor=src.tensor, offset=src.offset,
                                           ap=[src.ap[0], [0, 64], src.ap[1]])
                            nc.sync.dma_start(out=pexp[:], in_=bsrc)
                            nc.vector.tensor_mul(
                                wt[ci][:, k * 512:(k + 1) * 512],
                                wt[ci][:, k * 512:(k + 1) * 512], pexp[:])
                    # spr[0,0:8] = S[h], spr[0,8:16] = PR[h] (read psS before
                    # the pool scope releases the bank)
                    nc.vector.reduce_sum(
                        spr[:].rearrange("p (g h) -> p g h", g=2),
                        psS[:].rearrange("p (g t h) -> p g h t", g=2, t=TPC, h=H),
                        axis=mybir.AxisListType.X)

            def global_scalars():
                with tc.tile_pool(name="gs", bufs=2, space="PSUM") as psg:
                    srec = small.tile([1, H], f32, tag="srec", name="srec")
                    nc.vector.tensor_scalar_add(srec[:], spr[0:1, 0:H], 1e-8)
                    nc.vector.reciprocal(srec[:], srec[:])
                    dots = small.tile([1, H], f32, tag="dots", name="dots")
                    nc.vector.tensor_mul(dots[:], spr[0:1, H:2 * H], srec[:])
                    nc.vector.tensor_scalar_add(dots[:], dots[:], 1.0)
                    nc.vector.reciprocal(watn[:], dots[:])
                    nc.vector.tensor_scalar_mul(watn[:], watn[:], -1.0)
                    psW = psg.tile([H, 1], f32, tag="psW", name="psW")
                    nc.tensor.matmul(psW[:], watn[:], idf[:], is_transpose=True)
                    nc.scalar.activation(out=watnT[:], in_=psW[:], func=Act.Copy)
                    # wo[ci] *= -attn[h(c)] (per-partition) via a tiny expand matmul
                    for ci in range(CT):
                        psWE = psg.tile([128, 1], f32, tag="psWE", name="psWE")
                        nc.tensor.matmul(
                            psWE[:], ind8_sb[:, ci * 128:(ci + 1) * 128],
                            watnT[:], start=True, stop=True)
                        wex = small.tile([128, 1], f32, tag="wex", name="wex")
                        nc.scalar.activation(out=wex[:], in_=psWE[:], func=Act.Copy)
                        nc.vector.tensor_scalar_mul(wo[ci][:], wo[ci][:], wex[:])

            def phase3():
                with (
                    tc.tile_pool(name="p3o", bufs=6) as op,
                    tc.tile_pool(name="p3ps", bufs=6, space="PSUM") as psp,
                ):
                    for k in range(NCH):
                        for oj in range(CT):
                            psC = psp.tile([128, 512], f32, tag="psC", name="psC")
                            for ci in range(CT):
                                nc.tensor.matmul(
                                    psC[:], wo[ci][:, oj * 128:(oj + 1) * 128],
                                    wt[ci][:, k * 512:(k + 1) * 512],
                                    start=(ci == 0), stop=(ci == CT - 1))
                            oc = op.tile([128, 512], bf16, tag="outc", name="outc")
                            if oj % 2 == 0:
                                nc.scalar.activation(
                                    out=oc[:], in_=psC[:], func=Act.Identity,
                                    bias=bout_sb[:, oj:oj + 1], scale=1.0)
                            else:
                                nc.vector.tensor_scalar_add(
                                    oc[:], psC[:], bout_sb[:, oj:oj + 1])
                            nc.sync.dma_start(
                                out=outT[oj * 128:(oj + 1) * 128, k * 512:(k + 1) * 512],
                                in_=oc[:])

            for _rep in range(reps):
                if 1 in phases:
                    phase1()
                    norm_finalize()
                if 2 in phases:
                    phase2()
                if 3 in phases:
                    global_scalars()
                    phase3()

    nc.compile()
    return nc


def _prep_inputs(x, token_mask, Wqkv, temp, Wout, bout):
    import ml_dtypes
    f = np.float32
    bf = ml_dtypes.bfloat16
    temp = np.asarray(temp, dtype=f)
    wqkvb = np.ascontiguousarray(np.asarray(Wqkv, f).T.astype(bf))
    woutb = np.ascontiguousarray(np.asarray(Wout, f).T.astype(bf))
    boutT = np.ascontiguousarray(np.asarray(bout, f).reshape(CT, 128).T)
    identb = np.eye(128, dtype=bf)
    ind8b = (np.arange(C) // D == np.arange(H)[:, None]).astype(bf)
    # tempP[p, ci] = temp[2ci + (p>=64)]
    tempP = np.empty((128, CT), f)
    for ci in range(CT):
        tempP[0:64, ci] = temp[2 * ci, 0]
        tempP[64:128, ci] = temp[2 * ci + 1, 0]
    in_maps = []
    for b in range(B):
        m = np.asarray(token_mask[b], f)          # [N]
        mt = m.reshape(NT, 128).T.copy()          # [128, NT]
        # per-partition mask bias; exact when temp == ones (the fixed fill)
        mb = (mt - 1.0) * (-MASK_NEG)
        in_maps.append({
            "xbf": np.ascontiguousarray(np.asarray(x[b], f).T.astype(bf)),
            "wqkvb": wqkvb,
            "woutb": woutb,
            "boutT": boutT,
            "maskf": mt,
            "mbias8": np.ascontiguousarray(mb),
            "tempP": tempP,
            "identb": identb,
            "ind8b": ind8b,
        })
    return in_maps


def kernel(**inputs):
    from concourse.bass_utils import run_bass_kernel_spmd

    if "nc" not in _CACHE:
        _CACHE["nc"] = _build_bass()
    nc = _CACHE["nc"]
    in_maps = _prep_inputs(**inputs)
    try:
        res = run_bass_kernel_spmd(nc, in_maps, core_ids=list(range(B)))
    except Exception:
        # transient device/tunnel hiccup: retry once
        import time as _t
        _t.sleep(2.0)
        res = run_bass_kernel_spmd(nc, in_maps, core_ids=list(range(B)))
    out = np.empty((B, N, C), np.float32)
    for b in range(B):
        out[b] = res.results[b]["outT"].T.astype(np.float32)
    return out


# revision 12
# speedup vs baseline: 3.1743x; 1.7156x over previous
"""Trainium2 Bass kernel for nn_AttentionTSSA (B=8, N=8192, C=512, H=8).

Sharding: data-parallel over batch B across the 8 NeuronCores (1 batch each,
no collectives).  bf16 data path throughout (inputs/outputs quantized on the
host); all matmul accumulation and the norm/softmax reductions stay f32.

Per core, three PE-paced stages with engine-balanced helpers:

  stage 1: wT[c,n] = Wqkv @ x^T in 512-token chunks (bf16 matmuls, f32 PSUM).
           Act evacuates PSUM -> wt (bf16); DVE tensor_tensor_reduce squares
           the same PSUM tile into w2 (bf16) AND emits the per-channel
           norm^2 partial in one pass.  DMA-in (bf16) on the SP queue.
  stage 2: per 128-token tile, a [c,16]-column matmul with the w2 tile as
           stationary produces (sum_ws | r) token-major.  Act computes
           exp(mask*logits + mbias) straight from PSUM (mask folded via
           scale/bias APs), DVE finishes the head-softmax; Pi and Pi*r are
           accumulated into the global S/PR bank with ones-matmuls.  Pi is
           transposed (PE) and broadcast to channel layout with a 0-stride
           SBUF DMA, then DVE scales wt in place (all bf16).
  stage 3: outT = Wout_scaled.T @ wt_scaled as a pure GEMM stream; -1/(1+dots)
           is folded into the wo weights.  Act/DVE alternate on the biased
           PSUM evacuation; DMA-out (bf16) on SP.

Host side: x is transposed/cast to bf16 per batch; outputs are cast back and
un-transposed.
"""

import numpy as np

B, N, C, H = 8, 8192, 512, 8
D = C // H          # 64
CT = C // 128       # 4 channel tiles
NCH = N // 512      # 16 chunks of 512 tokens
TPC = 4             # token tiles per chunk
NT = N // 128       # 64 token tiles

_CACHE = {}


def _build_bass(reps=1, phases=(1, 2, 3)):
    import concourse.bacc as bacc
    import concourse.bass as bass
    import concourse.mybir as mybir
    import concourse.tile as tile

    f32 = mybir.dt.float32
    bf16 = mybir.dt.bfloat16
    Alu = mybir.AluOpType
    Act = mybir.ActivationFunctionType

    nc = bacc.Bacc("TRN2", target_bir_lowering=False, debug=False, num_devices=B)

    xbf = nc.dram_tensor("xbf", [C, N], bf16, kind="ExternalInput")
    wqkvb = nc.dram_tensor("wqkvb", [C, C], bf16, kind="ExternalInput")
    woutb = nc.dram_tensor("woutb", [C, C], bf16, kind="ExternalInput")
    boutT = nc.dram_tensor("boutT", [128, CT], f32, kind="ExternalInput")
    maskf = nc.dram_tensor("maskf", [128, NT], f32, kind="ExternalInput")
    m8q = nc.dram_tensor("m8q", [128, NT], f32, kind="ExternalInput")
    tempP = nc.dram_tensor("tempP", [128, CT], f32, kind="ExternalInput")
    identb = nc.dram_tensor("identb", [128, 128], bf16, kind="ExternalInput")
    ind8b = nc.dram_tensor("ind8b", [H, C], bf16, kind="ExternalInput")
    outT = nc.dram_tensor("outT", [C, N], bf16, kind="ExternalOutput")

    with tile.TileContext(nc) as tc:
        with (
            tc.tile_pool(name="singles", bufs=1) as sing,
            tc.tile_pool(name="small", bufs=2) as small,
        ):
            # ---------------- constants / persistent tiles ----------------
            wq = [sing.tile([128, C], bf16, tag=f"wq{i}", name=f"wq{i}") for i in range(CT)]
            wo = [sing.tile([128, C], bf16, tag=f"wo{i}", name=f"wo{i}") for i in range(CT)]
            wt = [sing.tile([128, N], bf16, tag=f"wt{i}", name=f"wt{i}") for i in range(CT)]
            w2 = [sing.tile([128, N], bf16, tag=f"w2_{i}", name=f"w2_{i}") for i in range(CT)]
            # phase-1-critical loads (wq) go on the SP queue ahead of the x
            # stream; everything else loads via the Activation DMA queue.
            for i in range(CT):
                nc.sync.dma_start(out=wq[i][:], in_=wqkvb[i * 128:(i + 1) * 128, :])
                nc.scalar.dma_start(out=wo[i][:], in_=woutb[i * 128:(i + 1) * 128, :])
            bout_sb = sing.tile([128, CT], f32, tag="bout", name="bout")
            nc.scalar.dma_start(out=bout_sb[:], in_=boutT[:])
            maskf_sb = sing.tile([128, NT], f32, tag="maskf", name="maskf")
            nc.scalar.dma_start(out=maskf_sb[:], in_=maskf[:])
            m8q_sb = sing.tile([128, NT], f32, tag="m8q", name="m8q")
            nc.scalar.dma_start(out=m8q_sb[:], in_=m8q[:])
            tempP_sb = sing.tile([128, CT], f32, tag="tempP", name="tempP")
            nc.scalar.dma_start(out=tempP_sb[:], in_=tempP[:])
            ident_sb = sing.tile([128, 128], bf16, tag="ident", name="ident")
            nc.scalar.dma_start(out=ident_sb[:], in_=identb[:])
            ind8_sb = sing.tile([H, C], bf16, tag="ind8", name="ind8")
            nc.scalar.dma_start(out=ind8_sb[:], in_=ind8b[:])

            ones1 = sing.tile([128, 1], bf16, tag="ones1", name="ones1")
            nc.vector.memset(ones1[:], 1.0)
            idf = sing.tile([1, 1], f32, tag="idf", name="idf")
            nc.vector.memset(idf[:], 1.0)

            nrm = [sing.tile([128, NCH], f32, tag=f"nrm{i}", name=f"nrm{i}") for i in range(CT)]
            pi_all = sing.tile([128, NCH, TPC, H], bf16, tag="pi_all", name="pi_all")
            amat = [sing.tile([128, 16], bf16, tag=f"amat{i}", name=f"amat{i}") for i in range(CT)]
            inv = [sing.tile([128, 1], f32, tag=f"inv{i}", name=f"inv{i}") for i in range(CT)]
            spr = sing.tile([1, 16], f32, tag="spr", name="spr")
            watn = sing.tile([1, H], f32, tag="watn", name="watn")
            watnT = sing.tile([H, 1], bf16, tag="watnT", name="watnT")

            def phase1():
                with (
                    tc.tile_pool(name="p1x", bufs=8) as xp,
                    tc.tile_pool(name="p1ps", bufs=6, space="PSUM") as psp,
                ):
                    for k in range(NCH):
                        xt = []
                        for ci in range(CT):
                            t = xp.tile([128, 512], bf16, tag="xt", name="xt")
                            nc.sync.dma_start(
                                out=t[:],
                                in_=xbf[ci * 128:(ci + 1) * 128, k * 512:(k + 1) * 512])
                            xt.append(t)
                        for co in range(CT):
                            psA = psp.tile([128, 512], f32, tag="psA", name="psA")
                            for ci in range(CT):
                                nc.tensor.matmul(
                                    psA[:], wq[ci][:, co * 128:(co + 1) * 128], xt[ci][:],
                                    start=(ci == 0), stop=(ci == CT - 1))
                            nc.scalar.activation(
                                out=wt[co][:, k * 512:(k + 1) * 512], in_=psA[:],
                                func=Act.Copy)
                            # w2 = psA^2 (bf16) and norm^2 partial in one DVE pass
                            nc.vector.tensor_tensor_reduce(
                                out=w2[co][:, k * 512:(k + 1) * 512],
                                in0=psA[:], in1=psA[:], scale=1.0, scalar=0.0,
                                op0=Alu.mult, op1=Alu.add,
                                accum_out=nrm[co][:, k:k + 1])

            def norm_finalize():
                for ci in range(CT):
                    nsq = small.tile([128, 1], f32, tag="nsq_f", name="nsq_f")
                    nc.vector.reduce_sum(nsq[:], nrm[ci][:], axis=mybir.AxisListType.X)
                    nc.vector.tensor_scalar_max(nsq[:], nsq[:], 1e-24)
                    nc.vector.reciprocal(inv[ci][:], nsq[:])
                    am = small.tile([128, 16], f32, tag="am_f", name="am_f")
                    nc.vector.memset(am[:], 0.0)
                    # col 2ci (rows 0:64) / col 2ci+1 (rows 64:128): inv * temp
                    nc.vector.tensor_copy(am[0:64, 2 * ci:2 * ci + 1], inv[ci][0:64, :])
                    nc.vector.tensor_copy(am[64:128, 2 * ci + 1:2 * ci + 2], inv[ci][64:128, :])
                    nc.vector.tensor_scalar_mul(
                        am[:, 0:H], am[:, 0:H], tempP_sb[:, ci:ci + 1])
                    nc.vector.memset(am[0:64, 8 + 2 * ci:8 + 2 * ci + 1], 1.0)
                    nc.vector.memset(am[64:128, 8 + 2 * ci + 1:8 + 2 * ci + 2], 1.0)
                    nc.vector.tensor_copy(amat[ci][:], am[:])

            def phase2():
                with (
                    tc.tile_pool(name="p2w", bufs=3) as wp,
                    tc.tile_pool(name="p2psB", bufs=3, space="PSUM") as psb,
                    tc.tile_pool(name="p2psT", bufs=2, space="PSUM") as pst,
                    tc.tile_pool(name="p2psS", bufs=1, space="PSUM") as pss,
                ):
                    psS = pss.tile([1, 2 * TPC * H], f32, tag="psS", name="psS")
                    for k in range(NCH):
                        # one PSUM bank holds all 4 token tiles' (sum_ws | r)
                        pb = psb.tile([128, TPC, 16], f32, tag="psB", name="psB")
                        for ti in range(TPC):
                            t = k * TPC + ti
                            for ci in range(CT):
                                nc.tensor.matmul(
                                    pb[:, ti, :], w2[ci][:, t * 128:(t + 1) * 128],
                                    amat[ci][:],
                                    start=(ci == 0), stop=(ci == CT - 1))
                        # head softmax, token-major; masked tokens blend to the
                        # exact uniform 1/8 via erec*mask and +(1-mask)/8
                        ee = wp.tile([128, TPC, H], f32, tag="ee", name="ee")
                        nc.scalar.activation(out=ee[:], in_=pb[:, :, 0:H], func=Act.Exp)
                        erec = wp.tile([128, TPC], f32, tag="erec", name="erec")
                        nc.vector.reduce_sum(erec[:], ee[:], axis=mybir.AxisListType.X)
                        nc.vector.reciprocal(erec[:], erec[:])
                        nc.vector.tensor_mul(
                            erec[:], erec[:], maskf_sb[:, k * TPC:(k + 1) * TPC])
                        for ti in range(TPC):
                            t = k * TPC + ti
                            nc.vector.tensor_scalar(
                                out=pi_all[:, k, ti, :], in0=ee[:, ti, :],
                                scalar1=erec[:, ti:ti + 1], scalar2=m8q_sb[:, t:t + 1],
                                op0=Alu.mult, op1=Alu.add)
                        pirt = wp.tile([128, TPC, H], bf16, tag="pirt", name="pirt")
                        nc.vector.tensor_mul(
                            pirt[:], pi_all[:, k, :, :], pb[:, :, 8:16])
                        # global S / PR accumulators: one bank, two column groups
                        nc.tensor.matmul(
                            psS[0:1, 0:TPC * H], ones1[:], pi_all[:, k, :, :],
                            start=(k == 0), stop=(k == NCH - 1))
                        nc.tensor.matmul(
                            psS[0:1, TPC * H:2 * TPC * H], ones1[:], pirt[:],
                            start=(k == 0), stop=(k == NCH - 1))
                        # Pi back to channel layout: PE transpose, then 0-stride
                        # broadcast DMAs straight from PSUM (SP + Act queues)
                        psT = pst.tile([H, 512], bf16, tag="psT", name="psT")
                        for ti in range(TPC):
                            nc.tensor.transpose(
                                psT[:, ti * 128:(ti + 1) * 128],
                                pi_all[:, k, ti, :], ident_sb[:])
                        pitc = wp.tile([H, 512], bf16, tag="pitc", name="pitc")
                        nc.scalar.activation(out=pitc[:], in_=psT[:], func=Act.Copy)
                        for ci in range(CT):
                            eng = nc.sync if ci < 3 else nc.scalar
                            veng = nc.vector if ci % 2 == 0 else nc.gpsimd
                            pexp = wp.tile([128, 512], bf16, tag=f"pexp{ci}",
                                           name=f"pexp{ci}")
                            src = pitc[2 * ci:2 * ci + 2, :]
                            bsrc = bass.AP(tensor=src.tensor, offset=src.offset,
                                           ap=[src.ap[0], [0, 64], src.ap[1]])
                            eng.dma_start(out=pexp[:], in_=bsrc)
                            veng.tensor_mul(
                                wt[ci][:, k * 512:(k + 1) * 512],
                                wt[ci][:, k * 512:(k + 1) * 512], pexp[:])
                    # spr[0,0:8] = S[h], spr[0,8:16] = PR[h] (read psS before
                    # the pool scope releases the bank)
                    nc.vector.reduce_sum(
                        spr[:].rearrange("p (g h) -> p g h", g=2),
                        psS[:].rearrange("p (g t h) -> p g h t", g=2, t=TPC, h=H),
                        axis=mybir.AxisListType.X)

            def global_scalars():
                with tc.tile_pool(name="gs", bufs=2, space="PSUM") as psg:
                    srec = small.tile([1, H], f32, tag="srec", name="srec")
                    nc.vector.tensor_scalar_add(srec[:], spr[0:1, 0:H], 1e-8)
                    nc.vector.reciprocal(srec[:], srec[:])
                    dots = small.tile([1, H], f32, tag="dots", name="dots")
                    nc.vector.tensor_mul(dots[:], spr[0:1, H:2 * H], srec[:])
                    nc.vector.tensor_scalar_add(dots[:], dots[:], 1.0)
                    nc.vector.reciprocal(watn[:], dots[:])
                    nc.vector.tensor_scalar_mul(watn[:], watn[:], -1.0)
                    psW = psg.tile([H, 1], f32, tag="psW", name="psW")
                    nc.tensor.matmul(psW[:], watn[:], idf[:], is_transpose=True)
                    nc.scalar.activation(out=watnT[:], in_=psW[:], func=Act.Copy)
                    # wo[ci] *= -attn[h(c)] (per-partition) via a tiny expand matmul
                    for ci in range(CT):
                        psWE = psg.tile([128, 1], f32, tag="psWE", name="psWE")
                        nc.tensor.matmul(
                            psWE[:], ind8_sb[:, ci * 128:(ci + 1) * 128],
                            watnT[:], start=True, stop=True)
                        wex = small.tile([128, 1], f32, tag="wex", name="wex")
                        nc.scalar.activation(out=wex[:], in_=psWE[:], func=Act.Copy)
                        nc.vector.tensor_scalar_mul(wo[ci][:], wo[ci][:], wex[:])

            def phase3():
                with (
                    tc.tile_pool(name="p3o", bufs=6) as op,
                    tc.tile_pool(name="p3ps", bufs=6, space="PSUM") as psp,
                ):
                    for k in range(NCH):
                        for oj in range(CT):
                            psC = psp.tile([128, 512], f32, tag="psC", name="psC")
                            for ci in range(CT):
                                nc.tensor.matmul(
                                    psC[:], wo[ci][:, oj * 128:(oj + 1) * 128],
                                    wt[ci][:, k * 512:(k + 1) * 512],
                                    start=(ci == 0), stop=(ci == CT - 1))
                            oc = op.tile([128, 512], bf16, tag="outc", name="outc")
                            if oj % 2 == 0:
                                nc.scalar.activation(
                                    out=oc[:], in_=psC[:], func=Act.Identity,
                                    bias=bout_sb[:, oj:oj + 1], scale=1.0)
                            else:
                                nc.vector.tensor_scalar_add(
                                    oc[:], psC[:], bout_sb[:, oj:oj + 1])
                            nc.sync.dma_start(
                                out=outT[oj * 128:(oj + 1) * 128, k * 512:(k + 1) * 512],
                                in_=oc[:])

            for _rep in range(reps):
                if 1 in phases:
                    phase1()
                    norm_finalize()
                if 2 in phases:
                    phase2()
                if 3 in phases:
                    global_scalars()
                    phase3()

    nc.compile()
    return nc


def _prep_inputs(x, token_mask, Wqkv, temp, Wout, bout):
    import ml_dtypes
    f = np.float32
    bf = ml_dtypes.bfloat16
    temp = np.asarray(temp, dtype=f)
    wqkvb = np.ascontiguousarray(np.asarray(Wqkv, f).T.astype(bf))
    woutb = np.ascontiguousarray(np.asarray(Wout, f).T.astype(bf))
    boutT = np.ascontiguousarray(np.asarray(bout, f).reshape(CT, 128).T)
    identb = np.eye(128, dtype=bf)
    ind8b = (np.arange(C) // D == np.arange(H)[:, None]).astype(bf)
    # tempP[p, ci] = temp[2ci + (p>=64)]
    tempP = np.empty((128, CT), f)
    for ci in range(CT):
        tempP[0:64, ci] = temp[2 * ci, 0]
        tempP[64:128, ci] = temp[2 * ci + 1, 0]
    in_maps = []
    for b in range(B):
        m = np.asarray(token_mask[b], f)          # [N]
        mt = m.reshape(NT, 128).T.copy()          # [128, NT]
        in_maps.append({
            "xbf": np.ascontiguousarray(np.asarray(x[b], f).T.astype(bf)),
            "wqkvb": wqkvb,
            "woutb": woutb,
            "boutT": boutT,
            "maskf": mt,
            "m8q": np.ascontiguousarray((1.0 - mt) / 8.0),
            "tempP": tempP,
            "identb": identb,
            "ind8b": ind8b,
        })
    return in_maps


def kernel(**inputs):
    from concourse.bass_utils import run_bass_kernel_spmd

    if "nc" not in _CACHE:
        _CACHE["nc"] = _build_bass()
    nc = _CACHE["nc"]
    in_maps = _prep_inputs(**inputs)
    try:
        res = run_bass_kernel_spmd(nc, in_maps, core_ids=list(range(B)))
    except Exception:
        # transient device/tunnel hiccup: retry once
        import time as _t
        _t.sleep(2.0)
        res = run_bass_kernel_spmd(nc, in_maps, core_ids=list(range(B)))
    out = np.empty((B, N, C), np.float32)
    for b in range(B):
        out[b] = res.results[b]["outT"].T.astype(np.float32)
    return out
